# revision 31
# baseline (speedup 1.0000x reference)
"""Trainium2 8-core kernel for nn_CausalSelfAttention_11192684774089.

Computation (see reference): qkv = x@W_attn + b; LoRA on q,k; RoPE on q,k;
causal softmax attention; out = y@W_proj + b_proj.
  B=4, T=2048, C=2048, H=16 heads, D=128, fp32 I/O.

Sharding: tensor-parallel over heads (2 heads/core) for QKV + attention,
then an AllToAll switches to row-parallel for the output projection
(Megatron sequence-parallel style: A2A moves 4.2MB/core instead of a
67MB AllReduce). Host concatenates the 8 row-shards.

Device algorithm (per core):
  - LoRA is folded into effective weights on host: W_eff = W(I + s A B).
  - x is passed pre-transposed+bf16 [C, B*T]; QKV computed channel-major
    (q^T,k^T [128chan, ROWS]) and row-major for v, all SBUF-resident.
  - RoPE applied channel-major via a partition-permute DMA + 3 DVE ops,
    with host-precomputed cos/sin tables [128, T].
  - Attention in S^T layout: per (slice, head, batch) pass, key-tiles
    outer and the pass's two q-tiles inner.  Scores via PE into a paired
    2-bank PSUM tile, exp on ACT (fused 1/sqrt(D) scale) into a
    persistent pT buffer, causal mask via a 0/1 table multiply on DVE,
    softmax denominators accumulated on DVE (bf16) and partition-reduced
    with a single all-ones matmul that also broadcasts them to all 128
    partitions (no DMA round-trip), AV^T accumulated on the PE.
  - AllToAll (bf16) per (head, slice) -> y^T full-channel row-block;
    proj row-major + bias, with yt loads prefetched on the ACT DMA queue.
"""

import os
from contextlib import ExitStack

import numpy as np
import ml_dtypes

import concourse.bass as bass
import concourse.mybir as mybir
import concourse.tile as tile
from concourse import bacc
from concourse.bass_utils import run_bass_kernel_spmd

# This image's antenv lacks axon_hooks; run_bass_kernel_spmd(trace=True)
# imports it unconditionally. Register a working ctypes-based NTFF hook so
# tracing works (and doesn't crash) regardless of BASS_TRACE.
try:
    import antenv.axon_hooks  # noqa: F401
except ImportError:
    import sys as _sys
    import types as _types
    _hooks = _types.ModuleType("antenv.axon_hooks")
    try:
        from trn_agent_boot.trn_boot import _ntff_profile_via_ctypes
        _HOOK = _ntff_profile_via_ctypes("/opt/axon/libaxon_pjrt.so")
    except Exception:
        _HOOK = None
    _hooks.get_axon_ntff_profile_hook = lambda: _HOOK
    _hooks.set_axon_ntff_profile_hook = lambda h: None
    _sys.modules["antenv.axon_hooks"] = _hooks
    import concourse.bass_utils as _bu
    _orig_upload = _bu.upload_artifacts

    def _safe_upload(tmpdir):
        try:
            return _orig_upload(tmpdir)
        except Exception:
            return tmpdir

    _bu.upload_artifacts = _safe_upload

BF16 = ml_dtypes.bfloat16
FP32 = np.float32

# ---------------------------------------------------------------- config


class Cfg:
    def __init__(self, B=4, T=2048, C=2048, H=16, NC=8):
        self.B, self.T, self.C, self.H, self.NC = B, T, C, H, NC
        self.D = 128                      # head dim (fixed: RoPE tables assume 128)
        assert C == H * self.D
        self.H_LOC = H // NC              # heads per core
        self.OCQ = self.H_LOC * 128       # q chans per core
        self.OC = 3 * self.OCQ            # qkv chans per core
        self.ROWS = B * T
        self.RPC = self.ROWS // NC        # output rows per core
        self.KC = C // 128                # contraction chunks
        self.RT = 512                     # qkv row tile
        self.QT = 512                     # attention q tile
        self.KT = 128                     # attention key tile
        assert T % self.QT == 0 and self.RPC % 128 == 0
        assert self.QT % self.KT == 0 and self.ROWS % self.RT == 0
        self.SCALE = 1.0 / float(np.sqrt(self.D))
        # row-slice split for pipelined A2A+proj (needs 128-divisible halves)
        self.NSPL = 2 if (self.RPC // 2) % 128 == 0 else 1
        self.SPL = self.RPC // self.NSPL


CFG = Cfg()

# ---------------------------------------------------------------- builder


def build(cfg: Cfg, debug: bool = False):
    bf = mybir.dt.bfloat16
    f32 = mybir.dt.float32
    nc = bacc.Bacc(None, debug=debug, num_devices=cfg.NC)

    B, T, C, NC = cfg.B, cfg.T, cfg.C, cfg.NC
    H_LOC, OCQ, OC = cfg.H_LOC, cfg.OCQ, cfg.OC
    ROWS, RPC, KC, RT, QT, KT = cfg.ROWS, cfg.RPC, cfg.KC, cfg.RT, cfg.QT, cfg.KT
    NQK = 2 * H_LOC                       # number of q+k 128-chan blocks
    VOC = OCQ                             # v chans per core
    RB = ROWS // 128                      # v row blocks
    NKT = T // KT                         # key tiles per batch
    Ident = mybir.ActivationFunctionType.Identity
    Exp = mybir.ActivationFunctionType.Exp

    xT = nc.declare_dram_parameter("xT", [C, ROWS], bf, isOutput=False)
    w_eff = nc.declare_dram_parameter("w_eff", [C, OC], bf, isOutput=False)
    b_qk = nc.declare_dram_parameter("b_qk", [128, NQK], f32, isOutput=False)
    b_v = nc.declare_dram_parameter("b_v", [128, VOC], f32, isOutput=False)
    w_proj = nc.declare_dram_parameter("w_proj", [C, C], bf, isOutput=False)
    b_proj = nc.declare_dram_parameter("b_proj", [128, C], f32, isOutput=False)
    cosT = nc.declare_dram_parameter("cosT", [128, T], bf, isOutput=False)
    sinTs = nc.declare_dram_parameter("sinTs", [128, T], bf, isOutput=False)
    # 0/1 causal mask table: col m: 0 if m < 384 else (1 if m-384 >= p)
    t01 = nc.declare_dram_parameter("t01", [128, 512], bf, isOutput=False)
    out_ext = nc.declare_dram_parameter("out", [RPC, C], f32, isOutput=True)

    with tile.TileContext(nc) as tc, ExitStack() as top:
        const = top.enter_context(tc.tile_pool(name="const", bufs=1))
        dram = top.enter_context(tc.tile_pool(name="dram", bufs=1, space="DRAM"))

        # ---- constants in SBUF (cos/sin live in the phase-1 pool: they are
        # only needed for RoPE and freeing them makes room for proj weights)
        t01_sb = const.tile([128, 512], bf)
        bqk_sb = const.tile([128, NQK], f32)
        bv_sb = const.tile([128, VOC], f32)
        bproj_sb = const.tile([128, C], f32)
        ones_sb = const.tile([128, 1], bf)
        ones128 = const.tile([128, 128], bf)
        nc.sync.dma_start(bqk_sb[:], b_qk[:, :])
        nc.vector.memset(ones_sb[:], 1.0)
        nc.vector.memset(ones128[:], 1.0)
        warm_sb = const.tile([128, 128], bf)
        nc.vector.memset(warm_sb[:], 0.5)

        NSPL, SPL = cfg.NSPL, cfg.SPL
        a2a_in = [[dram.tile([NC, 128, SPL], bf, name=f"a2a_in_{h}_{s}")
                   for s in range(NSPL)] for h in range(H_LOC)]
        a2a_out = [[dram.tile([NC, 128, SPL], bf, name=f"a2a_out_{h}_{s}")
                    for s in range(NSPL)] for h in range(H_LOC)]
        qk_dram = dram.tile([128, 2 * H_LOC, ROWS], bf)

        # ---- persistent activation tiles (live into attention phase)
        act_pool = top.enter_context(tc.tile_pool(name="acts", bufs=1))
        qk_raw = act_pool.tile([128, NQK, ROWS], bf)     # q then k, chan-major
        v_sb = act_pool.tile([128, RB, VOC], bf)         # v row-major

        # attention softmax buffers, allocated BEFORE phase 1 so their NaN-
        # guard memsets run at t=0 on the idle DVE (allocating them later
        # would overlap freed phase-1 pools and add a false WAR on all of
        # QKV).  pT is split: paired-qt key-tiles [2*QT wide] + B-only tail
        # key-tiles [QT wide]; slot = kt (paired) / kt - nkA (tail).
        pT_p = act_pool.tile([128, 8, 2 * QT], bf)
        pT_t = act_pool.tile([128, 8, QT], bf)
        for sl, w in ((1, 128), (2, 256), (3, 384), (5, 128), (6, 256),
                      (7, 384)):
            nc.vector.memset(pT_p[:, sl, 0:w], 0.0)
        for sl, w in ((5, 128), (6, 256), (7, 384)):
            nc.vector.memset(pT_t[:, sl, 0:w], 0.0)

        # ========= Phase 1: QKV + fused RoPE (per row tile) =========
        qkd = qk_dram.rearrange("(hh two) o r -> two hh o r", two=2)
        with tc.tile_pool(name="qkv_w", bufs=1) as wpool, \
             tc.tile_pool(name="qkv_x", bufs=2) as xpool, \
             tc.tile_pool(name="rope_tmp", bufs=3) as tpool, \
             tc.tile_pool(name="qkv_ps", bufs=3, space="PSUM") as qps, \
             tc.tile_pool(name="qkv_psv", bufs=2, space="PSUM") as vps:
            w_sb = wpool.tile([128, KC, OC], bf)
            cos_sb = wpool.tile([128, T], bf, name="cos_sb")
            sin_sb = wpool.tile([128, T], bf, name="sin_sb")
            w_view = w_eff.rearrange("(kc p) oc -> p kc oc", p=128)
            for rt in range(ROWS // RT):
                rsl = slice(rt * RT, (rt + 1) * RT)
                tsl = slice((rt * RT) % T, (rt * RT) % T + RT)  # t within batch
                xt = xpool.tile([128, KC, RT], bf, name="xt")
                xt_view = xT[:, rsl].rearrange("(kc p) r -> p kc r", p=128)
                if rt == 0:   # first tile split so MMs can start early:
                    # x in two halves on sync, weights as one bulk DMA on
                    # scalar (single completion at ~8us beats 16 chunk
                    # arrivals trickling to ~17us)
                    nc.sync.dma_start(xt[:, 0:KC // 2, :],
                                      xt_view[:, 0:KC // 2, :])
                    nc.sync.dma_start(xt[:, KC // 2:, :],
                                      xt_view[:, KC // 2:, :])
                    nc.scalar.dma_start(w_sb[:], w_view[:, :, :])
                    # big constants: gpsimd queue, off the hot sync queue
                    nc.gpsimd.dma_start(cos_sb[:], cosT[:, :])
                    nc.gpsimd.dma_start(sin_sb[:], sinTs[:, :])
                    nc.gpsimd.dma_start(t01_sb[:], t01[:, :])
                    nc.gpsimd.dma_start(bv_sb[:], b_v[:, :])
                    nc.gpsimd.dma_start(bproj_sb[:], b_proj[:, :])
                    # HAM warm-up: ~6us of tiny matmuls while DMAs stream in
                    wps = qps.tile([1, 128], f32, name="warm_ps")
                    for _ in range(100):
                        nc.tensor.matmul(wps[:], lhsT=ones_sb[:],
                                         rhs=warm_sb[:], start=True, stop=True)
                else:
                    nc.sync.dma_start(xt[:], xt_view)
                for o in range(NQK):
                    ps = qps.tile([128, RT], f32, name="qk_ps")
                    for k in range(KC):
                        nc.tensor.matmul(
                            ps[:], lhsT=w_sb[:, k, o * 128:(o + 1) * 128],
                            rhs=xt[:, k, :], start=(k == 0), stop=(k == KC - 1))
                    nc.scalar.activation(
                        qk_raw[:, o, rsl], ps[:], Ident, bias=bqk_sb[:, o:o + 1])
                    # RoPE, fused at row-tile granularity:
                    nc.sync.dma_start(qk_dram[:, o, rsl], qk_raw[:, o, rsl])
                    tld = tpool.tile([128, RT], bf, name="tld")
                    nc.sync.dma_start(tld[0:64, :], qkd[1, :, o, rsl])
                    nc.sync.dma_start(tld[64:128, :], qkd[0, :, o, rsl])
                    nc.vector.tensor_mul(tld[:], tld[:], sin_sb[:, tsl])
                    nc.vector.tensor_mul(
                        qk_raw[:, o, rsl], qk_raw[:, o, rsl], cos_sb[:, tsl])
                    nc.vector.tensor_add(
                        qk_raw[:, o, rsl], qk_raw[:, o, rsl], tld[:])
                for rs in range(RT // 128):
                    psv = vps.tile([128, VOC], f32, name="v_ps")
                    for k in range(KC):
                        nc.tensor.matmul(
                            psv[:], lhsT=xt[:, k, rs * 128:(rs + 1) * 128],
                            rhs=w_sb[:, k, NQK * 128:], start=(k == 0),
                            stop=(k == KC - 1))
                    nc.vector.tensor_add(
                        v_sb[:, rt * (RT // 128) + rs, :], psv[:], bv_sb[:])

        # proj weights: half loads during attention so the DMA overlaps
        OT = 512
        CH = max(C // 2, OT)
        pwpool = top.enter_context(tc.tile_pool(name="proj_w", bufs=1))
        pw_sb = pwpool.tile([128, KC, CH], bf, name="pw_sb")
        nc.sync.dma_start(
            pw_sb[:], w_proj[:, 0:CH].rearrange("(kc p) oc -> p kc oc", p=128))
        pw_halves = [pw_sb]

        # proj input tiles: pool spans attention+proj so the first two
        # slice-0 row-blocks can prefetch on the (idle) sync queue during
        # late attention — their on-demand loads otherwise crawl against
        # the last AllToAll's HBM traffic (measured 16us for 512KB)
        KH = KC // H_LOC                  # contraction chunks per head-group
        ypool = top.enter_context(tc.tile_pool(name="proj_yt", bufs=2))
        yt_pre = []

        def load_yt(s_, rt, queue):
            yt = ypool.tile([128, KC, 128], bf, name="yt")
            for hh in range(H_LOC):
                view = a2a_out[hh][s_].rearrange("sl c r -> (sl c) r")
                queue.dma_start(
                    yt[:, hh * KH:(hh + 1) * KH, :],
                    view[:, rt * 128:(rt + 1) * 128]
                    .rearrange("(kc p) r -> p kc r", p=128))
            return yt

        # ================= Phase 3: attention + split A2A =================
        # Slice s holds rows (row % RPC) // SPL == s, i.e. q-tiles with
        # qt % 2 == s.  Each (s, h, b) pass runs key-tiles outer over the
        # two q-tiles {s, s+2}; A2A(h, s) fires once all b are done.
        LAG = 2
        with tc.tile_pool(name="attn_acc", bufs=1) as accpool, \
             tc.tile_pool(name="attn_rec", bufs=3) as rpool, \
             tc.tile_pool(name="attn_y", bufs=4) as ypool2, \
             tc.tile_pool(name="s_ps", bufs=2, space="PSUM") as sps, \
             tc.tile_pool(name="av_ps", bufs=2, space="PSUM") as avps:
            # dn accumulator, split even/odd so the two DVE add chains run
            # independently (no memset needed: kt 0/1 copy-initialize)
            acc = accpool.tile([128, 2, 2 * QT], bf)
            for s in range(NSPL):
                for h in range(H_LOC):
                    qh = qk_raw[:, h, :]
                    kh = qk_raw[:, H_LOC + h, :]
                    for b in range(B):
                        qts = (s, s + 2)
                        nks = [4 * (qt + 1) for qt in qts]
                        nkA, nkB = nks
                        av = avps.tile([128, 2 * QT], f32, name="av")

                        def pslice(kt, j, dd):
                            # pT slice for (key-tile, q-tile j) cols [dd:QT)
                            if kt < nkA:
                                return pT_p[:, kt, j * QT + dd:(j + 1) * QT]
                            return pT_t[:, kt - nkA, dd:QT]

                        def consume(kt):
                            # dn accumulate (DVE) + AV matmuls for key-tile kt
                            par = kt % 2
                            if kt < 2:
                                nc.vector.tensor_copy(acc[:, par, :],
                                                      pT_p[:, kt, :])
                            elif kt < nkA:
                                nc.vector.tensor_add(
                                    acc[:, par, :], acc[:, par, :],
                                    pT_p[:, kt, :])
                            else:
                                nc.vector.tensor_add(
                                    acc[:, par, QT:], acc[:, par, QT:],
                                    pT_t[:, kt - nkA, :])
                            for j, qt in enumerate(qts):
                                nk = nks[j]
                                if kt >= nk:
                                    continue
                                dd = max(0, (kt - 4 * qt) * KT)
                                nc.tensor.matmul(
                                    av[:, j * QT + dd:(j + 1) * QT],
                                    lhsT=v_sb[:, b * NKT + kt,
                                              h * 128:(h + 1) * 128],
                                    rhs=pslice(kt, j, dd),
                                    start=(kt == 0), stop=(kt == nk - 1))

                        sp_pair = [None]
                        for kt in range(nkB):
                            k0 = b * T + kt * KT
                            tail = kt - nkA
                            qB0 = b * T + (s + 2) * QT
                            if 0 <= tail < 4 and tail % 2 == 0:
                                # non-diag qB-only pair: defer exp so one
                                # ACT instruction covers two key-tiles
                                sp3 = sps.tile([128, 2, QT], f32, name="sp")
                                sp_pair[0] = sp3
                                nc.tensor.matmul(
                                    sp3[:, 0, :], lhsT=kh[:, k0:k0 + KT],
                                    rhs=qh[:, qB0:qB0 + QT],
                                    start=True, stop=True)
                                if kt >= LAG:
                                    consume(kt - LAG)
                                continue
                            if 0 <= tail < 4 and tail % 2 == 1:
                                sp3 = sp_pair[0]
                                nc.tensor.matmul(
                                    sp3[:, 1, :], lhsT=kh[:, k0:k0 + KT],
                                    rhs=qh[:, qB0:qB0 + QT],
                                    start=True, stop=True)
                                nc.scalar.activation(
                                    pT_t[:, tail - 1:tail + 1, :],
                                    sp3[:, :, :], Exp, scale=cfg.SCALE)
                                if kt >= LAG:
                                    consume(kt - LAG)
                                continue
                            sp = sps.tile([128, 2 * QT], f32, name="sp")
                            lo = None
                            for j, qt in enumerate(qts):
                                if kt >= nks[j]:
                                    continue
                                dd = max(0, kt * KT - qt * QT)
                                q0 = b * T + qt * QT
                                nc.tensor.matmul(
                                    sp[:, j * QT + dd:(j + 1) * QT],
                                    lhsT=kh[:, k0:k0 + KT],
                                    rhs=qh[:, q0 + dd:q0 + QT],
                                    start=True, stop=True)
                                if lo is None:
                                    lo = (j * QT + dd, dd)
                            # one exp over the contiguous valid span
                            if kt < nkA:
                                nc.scalar.activation(
                                    pT_p[:, kt, lo[0]:2 * QT],
                                    sp[:, lo[0]:2 * QT], Exp, scale=cfg.SCALE)
                            else:
                                nc.scalar.activation(
                                    pT_t[:, kt - nkA, lo[1]:QT],
                                    sp[:, lo[0]:2 * QT], Exp, scale=cfg.SCALE)
                            # causal 0/1 mask on diag tiles (also zeroes the
                            # skipped cols left of dd)
                            for j, qt in enumerate(qts):
                                if kt >= nks[j]:
                                    continue
                                dd = max(0, kt * KT - qt * QT)
                                if kt * KT + KT - 1 > qt * QT:
                                    tgt = pslice(kt, j, 0)
                                    nc.vector.tensor_mul(
                                        tgt[:, 0:dd + KT], tgt[:, 0:dd + KT],
                                        t01_sb[:, 384 - dd:512])
                            if kt >= LAG:
                                consume(kt - LAG)
                        for kt in range(max(0, nkB - LAG), nkB):
                            consume(kt)
                        # denominators: all-ones matmul partition-reduces acc
                        # and broadcasts the sums to every partition
                        dn = sps.tile([128, 2 * QT], f32, name="sp")
                        for par in range(2):   # par-outer: the even-chain
                            # MMs don't wait for the odd chain's last add
                            for j in range(2):
                                nc.tensor.matmul(
                                    dn[:, j * QT:(j + 1) * QT],
                                    lhsT=ones128[:],
                                    rhs=acc[:, par, j * QT:(j + 1) * QT],
                                    start=(par == 0), stop=(par == 1))
                        rec = rpool.tile([128, 2 * QT], f32, name="rec")
                        nc.vector.reciprocal_approx_fast(out=rec[:], in_=dn[:])
                        y_sb = ypool2.tile([128, 2 * QT], bf, name="y_sb")
                        nc.vector.tensor_mul(y_sb[:], av[:], rec[:])
                        for j, qt in enumerate(qts):
                            g = b * T + qt * QT
                            nc.sync.dma_start(
                                a2a_in[h][s][g // RPC, :, :],
                                y_sb[:, j * QT:(j + 1) * QT])
                    # rows of (head h, slice s) complete -> exchange them now
                    nc.gpsimd.collective_compute(
                        "AllToAll", mybir.AluOpType.bypass,
                        replica_groups=[list(range(NC))],
                        ins=[a2a_in[h][s][:].opt()],
                        outs=[a2a_out[h][s][:].opt()])
                    if s == NSPL - 1 and h == 0:
                        # slice-0 exchanges are done; prefetch proj inputs
                        for rt_ in range(2):
                            yt_pre.append(load_yt(0, rt_, nc.sync))

        # ================= Phase 4: proj (per row-slice) =================
        with tc.tile_pool(name="proj_w2", bufs=1) as pw2pool, \
             tc.tile_pool(name="proj_o", bufs=2) as opool, \
             tc.tile_pool(name="proj_ps", bufs=4, space="PSUM") as pps:
            for ch in range(1, C // CH):
                pw2 = pw2pool.tile([128, KC, CH], bf, name="pw2_sb")
                nc.gpsimd.dma_start(
                    pw2[:], w_proj[:, ch * CH:(ch + 1) * CH]
                    .rearrange("(kc p) oc -> p kc oc", p=128))
                pw_halves.append(pw2)
            visits = [(ch, s) for s in range(NSPL) for ch in range(C // CH)]
            for vi, (ch, s) in enumerate(visits):
                pw = pw_halves[ch]
                for rt in range(SPL // 128):
                    if vi == 0 and rt < 2:
                        yt = yt_pre[rt]
                    else:
                        yt = load_yt(s, rt, nc.scalar)
                    for ot in range(CH // OT):
                        oc0 = ch * CH + ot * OT
                        ps = pps.tile([128, OT], f32, name="o_ps")
                        for k in range(KC):
                            nc.tensor.matmul(
                                ps[:], lhsT=yt[:, k, :],
                                rhs=pw[:, k, ot * OT:(ot + 1) * OT],
                                start=(k == 0), stop=(k == KC - 1))
                        o_sb = opool.tile([128, OT], f32, name="o_sb")
                        nc.vector.tensor_add(
                            o_sb[:], ps[:], bproj_sb[:, oc0:oc0 + OT])
                        r0 = s * SPL + rt * 128
                        nc.sync.dma_start(
                            out_ext[r0:r0 + 128, oc0:oc0 + OT],
                            o_sb[:])

    nc.compile()
    return nc


# ---------------------------------------------------------------- host prep


def host_prep(cfg: Cfg, x, W_attn, b_attn, lora_A_q, lora_B_q, lora_A_k,
              lora_B_k, W_proj, b_proj, lora_scaling=0.125):
    """Returns (in_maps, assemble_fn)."""
    B, T, C, NC, D = cfg.B, cfg.T, cfg.C, cfg.NC, cfg.D
    s = lora_scaling
    W = np.asarray(W_attn, FP32)
    bb = np.asarray(b_attn, FP32)
    Wq, Wk, Wv = W[:, :C], W[:, C:2 * C], W[:, 2 * C:]
    bq, bk, bv = bb[:C], bb[C:2 * C], bb[2 * C:]
    Aq = np.asarray(lora_A_q, FP32); Bq = np.asarray(lora_B_q, FP32)
    Ak = np.asarray(lora_A_k, FP32); Bk = np.asarray(lora_B_k, FP32)
    Wq_eff = Wq + (Wq @ Aq) @ Bq * s
    Wk_eff = Wk + (Wk @ Ak) @ Bk * s
    bq_eff = bq + (bq @ Aq) @ Bq * s
    bk_eff = bk + (bk @ Ak) @ Bk * s

    xT = np.ascontiguousarray(
        np.asarray(x, FP32).reshape(cfg.ROWS, C).T).astype(BF16)

    inv = 1.0 / (10000.0 ** (np.arange(0, D, 2, dtype=FP32) / D))
    tt = np.arange(T, dtype=FP32)
    fr = np.outer(tt, inv)
    cos = np.cos(np.concatenate([fr, fr], axis=1)).T.astype(BF16).copy()  # [128,T]
    sin = np.sin(np.concatenate([fr, fr], axis=1)).T.astype(FP32)
    sin[:64] *= -1.0
    sinTs = sin.astype(BF16).copy()

    # 0/1 causal keep-mask table: col m < 384 -> 0 (skipped cols), else
    # keep iff (m - 384) >= p
    kk = np.arange(128)[:, None]
    mm = np.arange(512)[None, :]
    T01 = np.where((mm >= 384) & (mm - 384 >= kk), 1.0, 0.0).astype(BF16)

    # permute W_proj rows to match the a2a_out channel order:
    # for each head-slot h: core 0's head h, core 1's head h, ...
    perm = np.concatenate(
        [np.arange(cfg.OCQ * i + 128 * h, cfg.OCQ * i + 128 * (h + 1))
         for h in range(cfg.H_LOC) for i in range(NC)])
    Wp = np.asarray(W_proj, FP32)[perm].astype(BF16)
    bp_rep = np.ascontiguousarray(
        np.broadcast_to(np.asarray(b_proj, FP32)[None, :], (128, C)))

    in_maps = []
    for c in range(NC):
        cs = slice(cfg.OCQ * c, cfg.OCQ * (c + 1))
        W_eff_c = np.concatenate(
            [Wq_eff[:, cs], Wk_eff[:, cs], Wv[:, cs]], axis=1).astype(BF16)
        bqk_c = np.concatenate([bq_eff[cs], bk_eff[cs]])          # [2*OCQ]
        bqk_c = np.ascontiguousarray(
            bqk_c.reshape(2 * cfg.H_LOC, 128).T).astype(FP32)     # [128, NQK]
        bv_c = np.ascontiguousarray(
            np.broadcast_to(bv[cs][None, :], (128, cfg.OCQ))).astype(FP32)
        in_maps.append({
            "xT": xT, "w_eff": W_eff_c, "b_qk": bqk_c, "b_v": bv_c,
            "w_proj": Wp, "b_proj": bp_rep, "cosT": cos, "sinTs": sinTs,
            "t01": T01,
        })

    def assemble(results):
        out = np.concatenate([np.asarray(r["out"], FP32) for r in results], axis=0)
        return out.reshape(B, T, C)

    return in_maps, assemble


# ---------------------------------------------------------------- entry

_NC_CACHE = {}
LAST_RESULT = None


def kernel(x, W_attn, b_attn, lora_A_q, lora_B_q, lora_A_k, lora_B_k,
           W_proj, b_proj):
    global LAST_RESULT
    cfg = CFG
    if "full" not in _NC_CACHE:
        _NC_CACHE["full"] = build(cfg)
    nc = _NC_CACHE["full"]
    in_maps, assemble = host_prep(
        cfg, x, W_attn, b_attn, lora_A_q, lora_B_q, lora_A_k, lora_B_k,
        W_proj, b_proj)
    res = run_bass_kernel_spmd(nc, in_maps, core_ids=list(range(cfg.NC)))
    LAST_RESULT = res
    return assemble(res.results)


if __name__ == "__main__":
    nc = build(CFG, debug=True)
    print("build OK; instructions:",
          sum(len(b.instructions) for b in nc.main_func.blocks))


# revision 34
# speedup vs baseline: 1.0359x; 1.0359x over previous
"""Trainium2 8-core kernel for nn_CausalSelfAttention_11192684774089.

Computation (see reference): qkv = x@W_attn + b; LoRA on q,k; RoPE on q,k;
causal softmax attention; out = y@W_proj + b_proj.
  B=4, T=2048, C=2048, H=16 heads, D=128, fp32 I/O.

Sharding: tensor-parallel over heads (2 heads/core) for QKV + attention,
then an AllToAll switches to row-parallel for the output projection
(Megatron sequence-parallel style: A2A moves 4.2MB/core instead of a
67MB AllReduce). Host concatenates the 8 row-shards.

Device algorithm (per core):
  - LoRA is folded into effective weights on host: W_eff = W(I + s A B).
  - x is passed pre-transposed+bf16 [C, B*T]; QKV computed channel-major
    (q^T,k^T [128chan, ROWS]) and row-major for v, all SBUF-resident.
  - RoPE applied channel-major via a partition-permute DMA + 3 DVE ops,
    with host-precomputed cos/sin tables [128, T].
  - Attention in S^T layout: per (slice, head, batch) pass, key-tiles
    outer and the pass's two q-tiles inner.  Scores via PE into a paired
    2-bank PSUM tile, exp on ACT (fused 1/sqrt(D) scale) into a
    persistent pT buffer, causal mask via a 0/1 table multiply on DVE,
    softmax denominators accumulated on DVE (bf16) and partition-reduced
    with a single all-ones matmul that also broadcasts them to all 128
    partitions (no DMA round-trip), AV^T accumulated on the PE.
  - AllToAll (bf16) per (head, slice) -> y^T full-channel row-block;
    proj row-major + bias, with yt loads prefetched on the ACT DMA queue.
"""

import os
from contextlib import ExitStack

import numpy as np
import ml_dtypes

import concourse.bass as bass
import concourse.mybir as mybir
import concourse.tile as tile
from concourse import bacc
from concourse.bass_utils import run_bass_kernel_spmd

# This image's antenv lacks axon_hooks; run_bass_kernel_spmd(trace=True)
# imports it unconditionally. Register a working ctypes-based NTFF hook so
# tracing works (and doesn't crash) regardless of BASS_TRACE.
try:
    import antenv.axon_hooks  # noqa: F401
except ImportError:
    import sys as _sys
    import types as _types
    _hooks = _types.ModuleType("antenv.axon_hooks")
    try:
        from trn_agent_boot.trn_boot import _ntff_profile_via_ctypes
        _HOOK = _ntff_profile_via_ctypes("/opt/axon/libaxon_pjrt.so")
    except Exception:
        _HOOK = None
    _hooks.get_axon_ntff_profile_hook = lambda: _HOOK
    _hooks.set_axon_ntff_profile_hook = lambda h: None
    _sys.modules["antenv.axon_hooks"] = _hooks
    import concourse.bass_utils as _bu
    _orig_upload = _bu.upload_artifacts

    def _safe_upload(tmpdir):
        try:
            return _orig_upload(tmpdir)
        except Exception:
            return tmpdir

    _bu.upload_artifacts = _safe_upload

BF16 = ml_dtypes.bfloat16
FP32 = np.float32

# ---------------------------------------------------------------- config


class Cfg:
    def __init__(self, B=4, T=2048, C=2048, H=16, NC=8):
        self.B, self.T, self.C, self.H, self.NC = B, T, C, H, NC
        self.D = 128                      # head dim (fixed: RoPE tables assume 128)
        assert C == H * self.D
        self.H_LOC = H // NC              # heads per core
        self.OCQ = self.H_LOC * 128       # q chans per core
        self.OC = 3 * self.OCQ            # qkv chans per core
        self.ROWS = B * T
        self.RPC = self.ROWS // NC        # output rows per core
        self.KC = C // 128                # contraction chunks
        self.RT = 512                     # qkv row tile
        self.QT = 512                     # attention q tile
        self.KT = 128                     # attention key tile
        assert T % self.QT == 0 and self.RPC % 128 == 0
        assert self.QT % self.KT == 0 and self.ROWS % self.RT == 0
        self.SCALE = 1.0 / float(np.sqrt(self.D))
        # row-slice split for pipelined A2A+proj (needs 128-divisible halves)
        self.NSPL = 2 if (self.RPC // 2) % 128 == 0 else 1
        self.SPL = self.RPC // self.NSPL


CFG = Cfg()

# ---------------------------------------------------------------- builder


def build(cfg: Cfg, debug: bool = False):
    bf = mybir.dt.bfloat16
    f32 = mybir.dt.float32
    nc = bacc.Bacc(None, debug=debug, num_devices=cfg.NC)

    B, T, C, NC = cfg.B, cfg.T, cfg.C, cfg.NC
    H_LOC, OCQ, OC = cfg.H_LOC, cfg.OCQ, cfg.OC
    ROWS, RPC, KC, RT, QT, KT = cfg.ROWS, cfg.RPC, cfg.KC, cfg.RT, cfg.QT, cfg.KT
    NQK = 2 * H_LOC                       # number of q+k 128-chan blocks
    VOC = OCQ                             # v chans per core
    RB = ROWS // 128                      # v row blocks
    NKT = T // KT                         # key tiles per batch
    Ident = mybir.ActivationFunctionType.Identity
    Exp = mybir.ActivationFunctionType.Exp

    xT = nc.declare_dram_parameter("xT", [C, ROWS], bf, isOutput=False)
    w_eff = nc.declare_dram_parameter("w_eff", [C, OC], bf, isOutput=False)
    b_qk = nc.declare_dram_parameter("b_qk", [128, NQK], f32, isOutput=False)
    b_v = nc.declare_dram_parameter("b_v", [128, VOC], f32, isOutput=False)
    w_proj = nc.declare_dram_parameter("w_proj", [C, C], bf, isOutput=False)
    b_proj = nc.declare_dram_parameter("b_proj", [128, C], f32, isOutput=False)
    cosT = nc.declare_dram_parameter("cosT", [128, T], bf, isOutput=False)
    sinTs = nc.declare_dram_parameter("sinTs", [128, T], bf, isOutput=False)
    # 0/1 causal mask table: col m: 0 if m < 384 else (1 if m-384 >= p)
    t01 = nc.declare_dram_parameter("t01", [128, 512], bf, isOutput=False)
    out_ext = nc.declare_dram_parameter("out", [RPC, C], f32, isOutput=True)

    with tile.TileContext(nc) as tc, ExitStack() as top:
        const = top.enter_context(tc.tile_pool(name="const", bufs=1))
        dram = top.enter_context(tc.tile_pool(name="dram", bufs=1, space="DRAM"))

        # ---- constants in SBUF (cos/sin live in the phase-1 pool: they are
        # only needed for RoPE and freeing them makes room for proj weights)
        t01_sb = const.tile([128, 512], bf)
        bqk_sb = const.tile([128, NQK], f32)
        bv_sb = const.tile([128, VOC], f32)
        bproj_sb = const.tile([128, C], f32)
        ones_sb = const.tile([128, 1], bf)
        ones128 = const.tile([128, 128], bf)
        nc.sync.dma_start(bqk_sb[:], b_qk[:, :])
        nc.vector.memset(ones_sb[:], 1.0)
        nc.vector.memset(ones128[:], 1.0)
        warm_sb = const.tile([128, 128], bf)
        nc.vector.memset(warm_sb[:], 0.5)

        NSPL, SPL = cfg.NSPL, cfg.SPL
        a2a_in = [[dram.tile([NC, 128, SPL], bf, name=f"a2a_in_{h}_{s}")
                   for s in range(NSPL)] for h in range(H_LOC)]
        a2a_out = [[dram.tile([NC, 128, SPL], bf, name=f"a2a_out_{h}_{s}")
                    for s in range(NSPL)] for h in range(H_LOC)]
        qk_dram = dram.tile([128, 2 * H_LOC, ROWS], bf)

        # ---- persistent activation tiles (live into attention phase)
        act_pool = top.enter_context(tc.tile_pool(name="acts", bufs=1))
        qk_raw = act_pool.tile([128, NQK, ROWS], bf)     # q then k, chan-major
        v_sb = act_pool.tile([128, RB, VOC], bf)         # v row-major

        # attention softmax buffers, allocated BEFORE phase 1 so their NaN-
        # guard memsets run at t=0 on the idle DVE (allocating them later
        # would overlap freed phase-1 pools and add a false WAR on all of
        # QKV).  pT is split: paired-qt key-tiles [2*QT wide] + B-only tail
        # key-tiles [QT wide]; slot = kt (paired) / kt - nkA (tail).
        pT_p = act_pool.tile([128, 8, 2 * QT], bf)
        pT_t = act_pool.tile([128, 8, QT], bf)
        for sl, w in ((1, 128), (2, 256), (3, 384), (5, 128), (6, 256),
                      (7, 384)):
            nc.vector.memset(pT_p[:, sl, 0:w], 0.0)
        for sl, w in ((5, 128), (6, 256), (7, 384)):
            nc.vector.memset(pT_t[:, sl, 0:w], 0.0)

        # ========= Phase 1: QKV + fused RoPE (per row tile) =========
        qkd = qk_dram.rearrange("(hh two) o r -> two hh o r", two=2)
        with tc.tile_pool(name="qkv_w", bufs=1) as wpool, \
             tc.tile_pool(name="qkv_x", bufs=2) as xpool, \
             tc.tile_pool(name="rope_tmp", bufs=3) as tpool, \
             tc.tile_pool(name="qkv_ps", bufs=3, space="PSUM") as qps, \
             tc.tile_pool(name="qkv_psv", bufs=2, space="PSUM") as vps:
            w_sb = wpool.tile([128, KC, OC], bf)
            cos_sb = wpool.tile([128, T], bf, name="cos_sb")
            sin_sb = wpool.tile([128, T], bf, name="sin_sb")
            w_view = w_eff.rearrange("(kc p) oc -> p kc oc", p=128)
            for rt in range(ROWS // RT):
                rsl = slice(rt * RT, (rt + 1) * RT)
                tsl = slice((rt * RT) % T, (rt * RT) % T + RT)  # t within batch
                xt = xpool.tile([128, KC, RT], bf, name="xt")
                xt_view = xT[:, rsl].rearrange("(kc p) r -> p kc r", p=128)
                if rt == 0:   # chunked first tile so MMs can start early,
                    # balanced over all three DMA queues
                    QS = (nc.sync, nc.scalar, nc.gpsimd)
                    for k in range(KC):
                        QS[k % 3].dma_start(xt[:, k, :], xt_view[:, k, :])
                        QS[(k + 1) % 3].dma_start(
                            w_sb[:, k, :], w_view[:, k, :])
                    # big constants: gpsimd queue, off the hot sync queue
                    nc.gpsimd.dma_start(cos_sb[:], cosT[:, :])
                    nc.gpsimd.dma_start(sin_sb[:], sinTs[:, :])
                    nc.gpsimd.dma_start(t01_sb[:], t01[:, :])
                    nc.gpsimd.dma_start(bv_sb[:], b_v[:, :])
                    nc.gpsimd.dma_start(bproj_sb[:], b_proj[:, :])
                    # HAM warm-up: ~6us of tiny matmuls while DMAs stream in
                    wps = qps.tile([1, 128], f32, name="warm_ps")
                    for _ in range(100):
                        nc.tensor.matmul(wps[:], lhsT=ones_sb[:],
                                         rhs=warm_sb[:], start=True, stop=True)
                else:
                    nc.sync.dma_start(xt[:], xt_view)
                for o in range(NQK):
                    ps = qps.tile([128, RT], f32, name="qk_ps")
                    for k in range(KC):
                        nc.tensor.matmul(
                            ps[:], lhsT=w_sb[:, k, o * 128:(o + 1) * 128],
                            rhs=xt[:, k, :], start=(k == 0), stop=(k == KC - 1))
                    nc.scalar.activation(
                        qk_raw[:, o, rsl], ps[:], Ident, bias=bqk_sb[:, o:o + 1])
                    # RoPE, fused at row-tile granularity:
                    nc.sync.dma_start(qk_dram[:, o, rsl], qk_raw[:, o, rsl])
                    tld = tpool.tile([128, RT], bf, name="tld")
                    nc.sync.dma_start(tld[0:64, :], qkd[1, :, o, rsl])
                    nc.sync.dma_start(tld[64:128, :], qkd[0, :, o, rsl])
                    nc.vector.tensor_mul(tld[:], tld[:], sin_sb[:, tsl])
                    nc.vector.tensor_mul(
                        qk_raw[:, o, rsl], qk_raw[:, o, rsl], cos_sb[:, tsl])
                    nc.vector.tensor_add(
                        qk_raw[:, o, rsl], qk_raw[:, o, rsl], tld[:])
                for rs in range(RT // 128):
                    psv = vps.tile([128, VOC], f32, name="v_ps")
                    for k in range(KC):
                        nc.tensor.matmul(
                            psv[:], lhsT=xt[:, k, rs * 128:(rs + 1) * 128],
                            rhs=w_sb[:, k, NQK * 128:], start=(k == 0),
                            stop=(k == KC - 1))
                    nc.vector.tensor_add(
                        v_sb[:, rt * (RT // 128) + rs, :], psv[:], bv_sb[:])

        # proj weights: half loads during attention so the DMA overlaps
        OT = 512
        CH = max(C // 2, OT)
        pwpool = top.enter_context(tc.tile_pool(name="proj_w", bufs=1))
        pw_sb = pwpool.tile([128, KC, CH], bf, name="pw_sb")
        nc.sync.dma_start(
            pw_sb[:], w_proj[:, 0:CH].rearrange("(kc p) oc -> p kc oc", p=128))
        pw_halves = [pw_sb]

        # proj input tiles: pool spans attention+proj so the first two
        # slice-0 row-blocks can prefetch on the (idle) sync queue during
        # late attention — their on-demand loads otherwise crawl against
        # the last AllToAll's HBM traffic (measured 16us for 512KB)
        KH = KC // H_LOC                  # contraction chunks per head-group
        ypool = top.enter_context(tc.tile_pool(name="proj_yt", bufs=2))
        yt_pre = []

        def load_yt(s_, rt, queue):
            yt = ypool.tile([128, KC, 128], bf, name="yt")
            for hh in range(H_LOC):
                view = a2a_out[hh][s_].rearrange("sl c r -> (sl c) r")
                queue.dma_start(
                    yt[:, hh * KH:(hh + 1) * KH, :],
                    view[:, rt * 128:(rt + 1) * 128]
                    .rearrange("(kc p) r -> p kc r", p=128))
            return yt

        # ================= Phase 3: attention + split A2A =================
        # Slice s holds rows (row % RPC) // SPL == s, i.e. q-tiles with
        # qt % 2 == s.  Each (s, h, b) pass runs key-tiles outer over the
        # two q-tiles {s, s+2}; A2A(h, s) fires once all b are done.
        LAG = 2
        with tc.tile_pool(name="attn_acc", bufs=1) as accpool, \
             tc.tile_pool(name="attn_rec", bufs=3) as rpool, \
             tc.tile_pool(name="attn_y", bufs=4) as ypool2, \
             tc.tile_pool(name="s_ps", bufs=2, space="PSUM") as sps, \
             tc.tile_pool(name="av_ps", bufs=2, space="PSUM") as avps:
            # dn accumulator, split even/odd so the two DVE add chains run
            # independently (no memset needed: kt 0/1 copy-initialize)
            acc = accpool.tile([128, 2, 2 * QT], bf)
            for s in range(NSPL):
                for h in range(H_LOC):
                    qh = qk_raw[:, h, :]
                    kh = qk_raw[:, H_LOC + h, :]
                    for b in range(B):
                        qts = (s, s + 2)
                        nks = [4 * (qt + 1) for qt in qts]
                        nkA, nkB = nks
                        av = avps.tile([128, 2 * QT], f32, name="av")

                        def pslice(kt, j, dd):
                            # pT slice for (key-tile, q-tile j) cols [dd:QT)
                            if kt < nkA:
                                return pT_p[:, kt, j * QT + dd:(j + 1) * QT]
                            return pT_t[:, kt - nkA, dd:QT]

                        def consume(kt):
                            # dn accumulate (DVE) + AV matmuls for key-tile kt
                            par = kt % 2
                            if kt < 2:
                                nc.vector.tensor_copy(acc[:, par, :],
                                                      pT_p[:, kt, :])
                            elif kt < nkA:
                                nc.vector.tensor_add(
                                    acc[:, par, :], acc[:, par, :],
                                    pT_p[:, kt, :])
                            else:
                                nc.vector.tensor_add(
                                    acc[:, par, QT:], acc[:, par, QT:],
                                    pT_t[:, kt - nkA, :])
                            for j, qt in enumerate(qts):
                                nk = nks[j]
                                if kt >= nk:
                                    continue
                                dd = max(0, (kt - 4 * qt) * KT)
                                nc.tensor.matmul(
                                    av[:, j * QT + dd:(j + 1) * QT],
                                    lhsT=v_sb[:, b * NKT + kt,
                                              h * 128:(h + 1) * 128],
                                    rhs=pslice(kt, j, dd),
                                    start=(kt == 0), stop=(kt == nk - 1))

                        sp_pair = [None]
                        for kt in range(nkB):
                            k0 = b * T + kt * KT
                            tail = kt - nkA
                            qB0 = b * T + (s + 2) * QT
                            if 0 <= tail < 4 and tail % 2 == 0:
                                # non-diag qB-only pair: defer exp so one
                                # ACT instruction covers two key-tiles
                                sp3 = sps.tile([128, 2, QT], f32, name="sp")
                                sp_pair[0] = sp3
                                nc.tensor.matmul(
                                    sp3[:, 0, :], lhsT=kh[:, k0:k0 + KT],
                                    rhs=qh[:, qB0:qB0 + QT],
                                    start=True, stop=True)
                                if kt >= LAG:
                                    consume(kt - LAG)
                                continue
                            if 0 <= tail < 4 and tail % 2 == 1:
                                sp3 = sp_pair[0]
                                nc.tensor.matmul(
                                    sp3[:, 1, :], lhsT=kh[:, k0:k0 + KT],
                                    rhs=qh[:, qB0:qB0 + QT],
                                    start=True, stop=True)
                                nc.scalar.activation(
                                    pT_t[:, tail - 1:tail + 1, :],
                                    sp3[:, :, :], Exp, scale=cfg.SCALE)
                                if kt >= LAG:
                                    consume(kt - LAG)
                                continue
                            sp = sps.tile([128, 2 * QT], f32, name="sp")
                            lo = None
                            for j, qt in enumerate(qts):
                                if kt >= nks[j]:
                                    continue
                                dd = max(0, kt * KT - qt * QT)
                                q0 = b * T + qt * QT
                                nc.tensor.matmul(
                                    sp[:, j * QT + dd:(j + 1) * QT],
                                    lhsT=kh[:, k0:k0 + KT],
                                    rhs=qh[:, q0 + dd:q0 + QT],
                                    start=True, stop=True)
                                if lo is None:
                                    lo = (j * QT + dd, dd)
                            # one exp over the contiguous valid span
                            if kt < nkA:
                                nc.scalar.activation(
                                    pT_p[:, kt, lo[0]:2 * QT],
                                    sp[:, lo[0]:2 * QT], Exp, scale=cfg.SCALE)
                            else:
                                nc.scalar.activation(
                                    pT_t[:, kt - nkA, lo[1]:QT],
                                    sp[:, lo[0]:2 * QT], Exp, scale=cfg.SCALE)
                            # causal 0/1 mask on diag tiles (also zeroes the
                            # skipped cols left of dd)
                            for j, qt in enumerate(qts):
                                if kt >= nks[j]:
                                    continue
                                dd = max(0, kt * KT - qt * QT)
                                if kt * KT + KT - 1 > qt * QT:
                                    tgt = pslice(kt, j, 0)
                                    nc.vector.tensor_mul(
                                        tgt[:, 0:dd + KT], tgt[:, 0:dd + KT],
                                        t01_sb[:, 384 - dd:512])
                            if kt >= LAG:
                                consume(kt - LAG)
                        for kt in range(max(0, nkB - LAG), nkB):
                            consume(kt)
                        # denominators: all-ones matmul partition-reduces acc
                        # and broadcasts the sums to every partition
                        dn = sps.tile([128, 2 * QT], f32, name="sp")
                        for par in range(2):   # par-outer: the even-chain
                            # MMs don't wait for the odd chain's last add
                            for j in range(2):
                                nc.tensor.matmul(
                                    dn[:, j * QT:(j + 1) * QT],
                                    lhsT=ones128[:],
                                    rhs=acc[:, par, j * QT:(j + 1) * QT],
                                    start=(par == 0), stop=(par == 1))
                        rec = rpool.tile([128, 2 * QT], f32, name="rec")
                        nc.vector.reciprocal_approx_fast(out=rec[:], in_=dn[:])
                        y_sb = ypool2.tile([128, 2 * QT], bf, name="y_sb")
                        nc.vector.tensor_mul(y_sb[:], av[:], rec[:])
                        for j, qt in enumerate(qts):
                            g = b * T + qt * QT
                            nc.sync.dma_start(
                                a2a_in[h][s][g // RPC, :, :],
                                y_sb[:, j * QT:(j + 1) * QT])
                    # rows of (head h, slice s) complete -> exchange them now
                    nc.gpsimd.collective_compute(
                        "AllToAll", mybir.AluOpType.bypass,
                        replica_groups=[list(range(NC))],
                        ins=[a2a_in[h][s][:].opt()],
                        outs=[a2a_out[h][s][:].opt()])
                    if s == NSPL - 1 and h == 0:
                        # slice-0 exchanges are done; prefetch proj inputs
                        for rt_ in range(2):
                            yt_pre.append(load_yt(0, rt_, nc.sync))

        # ================= Phase 4: proj (per row-slice) =================
        with tc.tile_pool(name="proj_w2", bufs=1) as pw2pool, \
             tc.tile_pool(name="proj_o", bufs=2) as opool, \
             tc.tile_pool(name="proj_ps", bufs=4, space="PSUM") as pps:
            for ch in range(1, C // CH):
                pw2 = pw2pool.tile([128, KC, CH], bf, name="pw2_sb")
                nc.gpsimd.dma_start(
                    pw2[:], w_proj[:, ch * CH:(ch + 1) * CH]
                    .rearrange("(kc p) oc -> p kc oc", p=128))
                pw_halves.append(pw2)
            visits = [(ch, s) for s in range(NSPL) for ch in range(C // CH)]
            for vi, (ch, s) in enumerate(visits):
                pw = pw_halves[ch]
                for rt in range(SPL // 128):
                    if vi == 0 and rt < 2:
                        yt = yt_pre[rt]
                    else:
                        yt = load_yt(s, rt, nc.scalar)
                    for ot in range(CH // OT):
                        oc0 = ch * CH + ot * OT
                        ps = pps.tile([128, OT], f32, name="o_ps")
                        for k in range(KC):
                            nc.tensor.matmul(
                                ps[:], lhsT=yt[:, k, :],
                                rhs=pw[:, k, ot * OT:(ot + 1) * OT],
                                start=(k == 0), stop=(k == KC - 1))
                        o_sb = opool.tile([128, OT], f32, name="o_sb")
                        nc.vector.tensor_add(
                            o_sb[:], ps[:], bproj_sb[:, oc0:oc0 + OT])
                        r0 = s * SPL + rt * 128
                        nc.sync.dma_start(
                            out_ext[r0:r0 + 128, oc0:oc0 + OT],
                            o_sb[:])

    nc.compile()
    return nc


# ---------------------------------------------------------------- host prep


def host_prep(cfg: Cfg, x, W_attn, b_attn, lora_A_q, lora_B_q, lora_A_k,
              lora_B_k, W_proj, b_proj, lora_scaling=0.125):
    """Returns (in_maps, assemble_fn)."""
    B, T, C, NC, D = cfg.B, cfg.T, cfg.C, cfg.NC, cfg.D
    s = lora_scaling
    W = np.asarray(W_attn, FP32)
    bb = np.asarray(b_attn, FP32)
    Wq, Wk, Wv = W[:, :C], W[:, C:2 * C], W[:, 2 * C:]
    bq, bk, bv = bb[:C], bb[C:2 * C], bb[2 * C:]
    Aq = np.asarray(lora_A_q, FP32); Bq = np.asarray(lora_B_q, FP32)
    Ak = np.asarray(lora_A_k, FP32); Bk = np.asarray(lora_B_k, FP32)
    Wq_eff = Wq + (Wq @ Aq) @ Bq * s
    Wk_eff = Wk + (Wk @ Ak) @ Bk * s
    bq_eff = bq + (bq @ Aq) @ Bq * s
    bk_eff = bk + (bk @ Ak) @ Bk * s

    xT = np.ascontiguousarray(
        np.asarray(x, FP32).reshape(cfg.ROWS, C).T).astype(BF16)

    inv = 1.0 / (10000.0 ** (np.arange(0, D, 2, dtype=FP32) / D))
    tt = np.arange(T, dtype=FP32)
    fr = np.outer(tt, inv)
    cos = np.cos(np.concatenate([fr, fr], axis=1)).T.astype(BF16).copy()  # [128,T]
    sin = np.sin(np.concatenate([fr, fr], axis=1)).T.astype(FP32)
    sin[:64] *= -1.0
    sinTs = sin.astype(BF16).copy()

    # 0/1 causal keep-mask table: col m < 384 -> 0 (skipped cols), else
    # keep iff (m - 384) >= p
    kk = np.arange(128)[:, None]
    mm = np.arange(512)[None, :]
    T01 = np.where((mm >= 384) & (mm - 384 >= kk), 1.0, 0.0).astype(BF16)

    # permute W_proj rows to match the a2a_out channel order:
    # for each head-slot h: core 0's head h, core 1's head h, ...
    perm = np.concatenate(
        [np.arange(cfg.OCQ * i + 128 * h, cfg.OCQ * i + 128 * (h + 1))
         for h in range(cfg.H_LOC) for i in range(NC)])
    Wp = np.asarray(W_proj, FP32)[perm].astype(BF16)
    bp_rep = np.ascontiguousarray(
        np.broadcast_to(np.asarray(b_proj, FP32)[None, :], (128, C)))

    in_maps = []
    for c in range(NC):
        cs = slice(cfg.OCQ * c, cfg.OCQ * (c + 1))
        W_eff_c = np.concatenate(
            [Wq_eff[:, cs], Wk_eff[:, cs], Wv[:, cs]], axis=1).astype(BF16)
        bqk_c = np.concatenate([bq_eff[cs], bk_eff[cs]])          # [2*OCQ]
        bqk_c = np.ascontiguousarray(
            bqk_c.reshape(2 * cfg.H_LOC, 128).T).astype(FP32)     # [128, NQK]
        bv_c = np.ascontiguousarray(
            np.broadcast_to(bv[cs][None, :], (128, cfg.OCQ))).astype(FP32)
        in_maps.append({
            "xT": xT, "w_eff": W_eff_c, "b_qk": bqk_c, "b_v": bv_c,
            "w_proj": Wp, "b_proj": bp_rep, "cosT": cos, "sinTs": sinTs,
            "t01": T01,
        })

    def assemble(results):
        out = np.concatenate([np.asarray(r["out"], FP32) for r in results], axis=0)
        return out.reshape(B, T, C)

    return in_maps, assemble


# ---------------------------------------------------------------- entry

_NC_CACHE = {}
LAST_RESULT = None


def kernel(x, W_attn, b_attn, lora_A_q, lora_B_q, lora_A_k, lora_B_k,
           W_proj, b_proj):
    global LAST_RESULT
    cfg = CFG
    if "full" not in _NC_CACHE:
        _NC_CACHE["full"] = build(cfg)
    nc = _NC_CACHE["full"]
    in_maps, assemble = host_prep(
        cfg, x, W_attn, b_attn, lora_A_q, lora_B_q, lora_A_k, lora_B_k,
        W_proj, b_proj)
    res = run_bass_kernel_spmd(nc, in_maps, core_ids=list(range(cfg.NC)))
    LAST_RESULT = res
    return assemble(res.results)


if __name__ == "__main__":
    nc = build(CFG, debug=True)
    print("build OK; instructions:",
          sum(len(b.instructions) for b in nc.main_func.blocks))
